# revision 1
# baseline (speedup 1.0000x reference)
"""DDSL polygon NUFT kernel for Trainium2 (8 NeuronCores).

Computes F(omega) = -RES^2 * sum_e C_e*D_e * DivDiff(e^{-i s}; s0,s1,0) over a
128x65 rfft2 frequency grid, for B=2 batches of 512 polygon edges.

Per (element e, frequency f):
    s0 = v0 . omega_f,  s1 = v1 . omega_f          (PE matmul, fp32)
    r = s0-s1, p = (s0-s1)*s0*s1, q ~= 1/p         (DVE, consistent fp32)
    tmp_re = (cos(s0)*s1 - cos(s1)*s0 + r) * q
    tmp_im = (sin(s1)*s0 - sin(s0)*s1) * q
    F += CD_e * tmp                                 (PE matmul reduction)

sin via ACT Sin LUT after range reduction: k = round(s/2pi) via fp32 magic-add
(ACT), then a fused (s - k*c1) - k*c2 Cody-Waite cascade (custom DVE op). The
HW Sin LUT is accurate slightly beyond +-pi (measured ~1e-12 at pi+1e-4), which
covers the cascade's worst-case overshoot, so no wrap is needed.
cos via the half-angle identity: cos(s) = 1 - 2*sin^2(s/2) (s/2 always in
domain); the -cos(s1) the pipeline needs is 2*sin^2(s1/2) - 1.

Sharding: (batch, elem) space = 2*512 = 1024 elements -> 8 cores x 128
elements (exactly the 128 SBUF partitions). omega replicated. Each core emits
a partial F over all 8320 frequencies; host sums 4 partials per batch.
"""

import numpy as np

B = 2
NV = 512
RES = 128
FX, FY = 128, 65
NF = FX * FY                    # 8320
N_CORES = 8
ELEMS_PER_CORE = (B * NV) // N_CORES   # 128
CHUNK = 1024                    # frequency columns per pipeline chunk
MM_MAX = 512                    # max matmul free dim (one PSUM bank)

_MAGIC = float(np.float32(1.5 * 2**23))
_INV2PI = float(np.float32(1.0 / (2.0 * np.pi)))
# 2-term Cody-Waite split of 2*pi: c1 has 5 mantissa bits so k*c1 is exact
# for |k| <= 131; residual k*(2pi - c1 - c2) ~ 1.3e-9 is below the fp32 floor.
_C1 = float(np.float32(6.28125))
_C2 = float(np.float32(np.float64(2 * np.pi) - np.float64(6.28125)))
_SQRT2 = float(np.float32(np.sqrt(2.0)))

_CHUNKS = []
_off = 0
while _off < NF:
    _CHUNKS.append((_off, min(CHUNK, NF - _off)))
    _off += CHUNK
# smallest chunk first: warms the PE->ACT->DVE pipeline ~8x faster and the
# kernel tail drains a full-width chunk instead of waiting on a cold one
_CHUNKS.sort(key=lambda cw: cw[1])


def _omega_t():
    """[2, NF] fp32 omega grid, bit-identical to the reference's _omega()."""
    f0 = np.fft.fftfreq(RES, d=1.0 / RES)
    f1 = np.fft.rfftfreq(RES, d=1.0 / RES)
    om = np.stack(np.meshgrid(f0, f1, indexing="ij"), axis=-1).astype(np.float32)
    om[..., 0] *= 2.0 * np.pi / 1.0
    om[..., 1] *= 2.0 * np.pi / 1.0
    omt = np.ascontiguousarray(om.reshape(NF, 2).T)       # [2, NF]
    # Frequency 0 is (0,0): s0=s1=0 there -> p=0 -> NaN through the approx
    # reciprocal. The bin is overwritten with the DC term on the host anyway,
    # so give it a harmless nonzero dummy frequency instead.
    omt[:, 0] = omt[:, 1]
    return omt


def _register_custom_ops():
    """Register the fused DVE ops with the concourse custom-op tables."""
    import numpy as np
    import concourse.dve_ops as dops
    from concourse.dve_spec import C0, C1, C2, Spec, Src0, Src1, lower
    from concourse.dve_spec import _has_src1 as has_src1
    from concourse.dve_uop import DveOpSpec

    if "DDSL_FUSED_P" in dops._SUB_OPCODE_FOR_NAME:
        by_name = {op.name: op for op in dops.OPS}
        return (by_name["DDSL_FUSED_P"], by_name["DDSL_CODY_MAGIC"],
                by_name["DDSL_SQ_MUL"])

    def mk(name, body, ref):
        spec = Spec(body=body, reference=ref)
        shas = {}
        for ver in ("v3", "v4"):
            uops = lower(spec, ver=ver)
            shas[ver] = DveOpSpec(
                name=name, opcode=0, uops=uops, rd1_en=has_src1(spec)
            ).sha(ver)
        op = dops.DveOp(name, spec, subdim=False, uops_sha=shas)
        dops.OPS.append(op)
        dops._SUB_OPCODE_FOR_NAME[name] = (
            dops._CUSTOM_DVE_ROW_BASE + len(dops.OPS) - 1
        )
        dops.CUSTOM_DVE_SPECS[name] = spec
        return op

    fused_p = mk(
        "DDSL_FUSED_P",
        (Src0 - Src1) * Src0 * Src1,
        lambda in0, in1, c0, c1, c2: ((in0 - in1) * in0) * in1,
    )
    _k = Src1 - C0
    cody_magic = mk(
        "DDSL_CODY_MAGIC",
        (Src0 - _k * C1) - _k * C2,
        lambda in0, in1, c0, c1, c2: (in0 - (in1 - c0) * c1) - (in1 - c0) * c2,
    )
    sq_mul = mk(
        "DDSL_SQ_MUL",
        Src0 * Src0 * Src1 * C0,
        lambda in0, in1, c0, c1, c2: ((in0 * in0) * np.asarray(in1).reshape(in0.shape)) * c0,
    )
    return fused_p, cody_magic, sq_mul


def _build_program():
    import concourse.bacc as bacc
    import concourse.bass as bass
    import concourse.mybir as mybir
    from concourse.tile import TileContext

    f32 = mybir.dt.float32
    Alu = mybir.AluOpType
    Act = mybir.ActivationFunctionType

    FUSED_P, CODY_MAGIC, SQ_MUL = _register_custom_ops()

    nc = bacc.Bacc(None)
    v01t = nc.dram_tensor("v01t", [4, ELEMS_PER_CORE], f32, kind="ExternalInput")
    cds = nc.dram_tensor("cds", [ELEMS_PER_CORE, 1], f32, kind="ExternalInput")
    omt = nc.dram_tensor("omt", [2, NF], f32, kind="ExternalInput")
    fpart = nc.dram_tensor("fpart", [1, 2 * NF], f32, kind="ExternalOutput")

    P = ELEMS_PER_CORE

    with TileContext(nc) as tc:
        with (
            tc.tile_pool(name="const", bufs=1) as cpool,
            tc.tile_pool(name="work", bufs=2) as wpool,
            tc.tile_pool(name="dveint", bufs=1) as ipool,
            tc.tile_pool(name="dveint2", bufs=2) as ipool2,
            tc.tile_pool(name="fout", bufs=3) as fpool,
            tc.tile_pool(name="ps_s", bufs=4, space="PSUM") as ps_s,
            tc.tile_pool(name="ps_f", bufs=1, space="PSUM") as ps_f,
        ):
            v0_sb = cpool.tile([2, P], f32)
            v1_sb = cpool.tile([2, P], f32)
            cds_sb = cpool.tile([P, 1], f32)
            om_sb = cpool.tile([2, NF], f32)
            nc.sync.dma_start(v0_sb[:], v01t[0:2, :])
            nc.sync.dma_start(v1_sb[:], v01t[2:4, :])
            nc.sync.dma_start(cds_sb[:], cds[:])
            # per-chunk omega slices so chunk 0 can start immediately
            for c0, w in _CHUNKS:
                nc.sync.dma_start(om_sb[:, c0 : c0 + w], omt[:, c0 : c0 + w])

            for c0, w in _CHUNKS:
                # ---- phases via PE into 1-bank PSUM pieces, ACT-copied into
                # the [s0 | s1] SBUF tile as they land ----
                s2 = wpool.tile([P, 2 * w], f32, tag="s2")
                for half, lhsT_t in enumerate((v0_sb, v1_sb)):
                    for m0 in range(0, w, MM_MAX):
                        mw = min(MM_MAX, w - m0)
                        s_ps = ps_s.tile([P, MM_MAX], f32, tag="s2ps")
                        nc.tensor.matmul(
                            s_ps[:, :mw],
                            lhsT_t[:],
                            om_sb[:, c0 + m0 : c0 + m0 + mw],
                        )
                        nc.scalar.activation(
                            s2[:, half * w + m0 : half * w + m0 + mw],
                            s_ps[:, :mw],
                            Act.Copy,
                        )
                mh = wpool.tile([P, 2 * w], f32, tag="mh")
                nc.scalar.activation(
                    mh[:], s2[:], Act.Copy, bias=_MAGIC, scale=_INV2PI
                )
                # ---- DVE: fused cascade: s2r = (s2 - k*c1) - k*c2 ----
                s2r = wpool.tile([P, 2 * w], f32, tag="s2r")
                nc.vector._custom_dve(
                    CODY_MAGIC, out=s2r[:], in0=s2[:], in1=mh[:],
                    s0=_MAGIC, s1=_C1, imm2=_C2,
                )
                # ---- ACT: sin2 = [sin0|sin1]; u2 = 2*sin^2(s/2) ----
                sin2 = wpool.tile([P, 2 * w], f32, tag="sin2")
                nc.scalar.activation(sin2[:], s2r[:], Act.Sin)
                sh2 = wpool.tile([P, 2 * w], f32, tag="sh2")
                nc.scalar.activation(sh2[:], s2r[:], Act.Sin, scale=0.5)
                # ---- DVE: denominator p = (s0-s1)*s0*s1 ----
                pp = ipool.tile([P, w], f32, tag="pp")
                nc.vector._custom_dve(
                    FUSED_P, out=pp[:], in0=s2[:, :w], in1=s2[:, w:]
                )
                qq = ipool.tile([P, w], f32, tag="qq")
                nc.vector.reciprocal_approx_fast(out=qq[:], in_=pp[:])
                # ---- DVE: products against the [s1 | s0] swapped view ----
                _s2full = s2[:]
                _s2h1 = s2[:, w:]
                s2_swap = bass.AP(
                    _s2full.tensor, _s2h1.offset,
                    [_s2full.ap[0], [-w, 2], [1, w]],
                )
                prod2 = ipool2.tile([P, 4 * w], f32, tag="prod2")
                nc.vector._custom_dve(
                    SQ_MUL, out=prod2[:, : 2 * w], in0=sh2[:], in1=s2_swap,
                    s0=2.0,
                )  # [u0'*s1 | u1'*s0],  u' = 2 sin^2(s/2)
                nc.vector.tensor_tensor(
                    prod2[:, 2 * w :], sin2[:], s2_swap, Alu.mult
                )  # [sin0*s1 | sin1*s0]
                # ---- DVE: numerators via one strided diff:
                # re2 = u1'*s0 - u0'*s1 ; im = sin1*s0 - sin0*s1 ----
                re2im = ipool.tile([P, 2 * w], f32, tag="re2im")
                _p2 = prod2[:]
                hi_view = bass.AP(_p2.tensor, prod2[:, w:].offset,
                                  [_p2.ap[0], [2 * w, 2], [1, w]])
                lo_view = bass.AP(_p2.tensor, _p2.offset,
                                  [_p2.ap[0], [2 * w, 2], [1, w]])
                nc.vector.tensor_tensor(re2im[:], hi_view, lo_view, Alu.subtract)
                tmp = wpool.tile([P, 2 * w], f32, tag="tmp")
                _qfull = qq[:]
                q_rep = bass.AP(
                    _qfull.tensor, _qfull.offset,
                    [_qfull.ap[0], [0, 2], [1, w]],
                )
                nc.vector.tensor_tensor(tmp[:], re2im[:], q_rep, Alu.mult)
                # ---- PE: weighted reduction over elements ----
                fchunk = fpool.tile([1, 2 * CHUNK], f32, tag="fchunk")
                f_ps = ps_f.tile([1, 2 * CHUNK], f32, tag="fps")
                for m0 in range(0, 2 * w, MM_MAX):
                    mw = min(MM_MAX, 2 * w - m0)
                    nc.tensor.matmul(
                        f_ps[:, m0 : m0 + mw], cds_sb[:, 0:1], tmp[:, m0 : m0 + mw]
                    )
                nc.scalar.activation(fchunk[:, : 2 * w], f_ps[:, : 2 * w], Act.Copy)
                nc.sync.dma_start(
                    fpart[:, 2 * c0 : 2 * c0 + 2 * w], fchunk[:, : 2 * w]
                )
    nc.compile()
    return nc


_PROGRAM = None


def _get_program():
    global _PROGRAM
    if _PROGRAM is None:
        _PROGRAM = _build_program()
    return _PROGRAM


def kernel(V, E, D):
    V = np.asarray(V)
    E = np.asarray(E)
    D = np.asarray(D)
    assert V.shape == (B, NV, 2) and E.shape == (B, NV, 2) and D.shape == (B, NV, 1)

    omt = _omega_t()

    # Host-side gather + per-edge scalars (tiny: B*NV elements).
    v0 = np.take_along_axis(V, E[:, :, 0:1].astype(np.int64), axis=1)  # (B,NV,2)
    v1 = np.take_along_axis(V, E[:, :, 1:2].astype(np.int64), axis=1)
    v0 = v0.astype(np.float32)
    v1 = v1.astype(np.float32)
    # C = v0x*v1y - v0y*v1x with the reference's fp32 rounding order.
    C = (v0[:, :, 0] * v1[:, :, 1]).astype(np.float32) - (
        v0[:, :, 1] * v1[:, :, 0]
    ).astype(np.float32)
    CD = (C * D[:, :, 0].astype(np.float32)).astype(np.float32)   # (B, NV)
    # Degenerate edges (E0==E1): reference masks them to zero contribution.
    # Perturb v1 so r != 0 on-device (avoids 1/0), and zero the weight.
    degen = E[:, :, 0] == E[:, :, 1]
    if degen.any():
        v1 = v1.copy()
        # perturbation direction (0.5, 155/512): r = -2pi*(0.5*fx + (155/512)*fy)
        # = -2pi*(256*fx + 155*fy)/512 is nonzero for every nonzero integer
        # grid frequency, so the zero-weight dummy rows never produce 1/0.
        v1[degen] = v0[degen] + np.float32([0.5, 155.0 / 512.0])
    cds_w = CD * np.float32(-(RES**2))                            # exact (2^14)
    cds_w = np.where(degen, np.float32(0.0), cds_w).astype(np.float32)

    in_maps = []
    for c in range(N_CORES):
        b = c // (N_CORES // B)
        sl = slice(ELEMS_PER_CORE * (c % (N_CORES // B)),
                   ELEMS_PER_CORE * (c % (N_CORES // B) + 1))
        v01 = np.ascontiguousarray(
            np.stack([v0[b, sl, 0], v0[b, sl, 1], v1[b, sl, 0], v1[b, sl, 1]])
        ).astype(np.float32)                                       # [4, 128]
        in_maps.append({
            "v01t": v01,
            "cds": np.ascontiguousarray(cds_w[b, sl][:, None]),
            "omt": omt,
        })

    omf = omt.T                                                   # (NF, 2)

    from concourse.bass_utils import run_bass_kernel_spmd

    nc = _get_program()
    res = run_bass_kernel_spmd(nc, in_maps, core_ids=list(range(N_CORES)))

    # ---- host gather: sum partials, de-chunk, DC fix ----
    Fre = np.zeros((B, NF), np.float64)
    Fim = np.zeros((B, NF), np.float64)
    for c in range(N_CORES):
        b = c // (N_CORES // B)
        part = res.results[c]["fpart"][0].astype(np.float64)
        for c0, w in _CHUNKS:
            Fre[b, c0 : c0 + w] += part[2 * c0 : 2 * c0 + w]
            Fim[b, c0 : c0 + w] += part[2 * c0 + w : 2 * c0 + 2 * w]

    # Exact fp32 zeros in the denominators (s0==s1, s0==0, or s1==0 at some
    # (element, frequency)) are masked to zero contribution by the reference
    # but turn into NaN through the device's approximate reciprocal, poisoning
    # that bin's PE reduction. Detect poisoned bins in the device output and
    # recompute them here (rare: ~1 bin per input set).
    f32 = np.float32
    bad = ~np.isfinite(Fre) | ~np.isfinite(Fim)
    for b, f in np.argwhere(bad):
        s0c = (v0[b] @ omf[f]).astype(f32)
        s1c = (v1[b] @ omf[f]).astype(f32)
        rc = f32(s0c - s1c)
        pc = f32(f32(rc * s0c) * s1c)
        with np.errstate(divide="ignore", invalid="ignore"):
            qc = f32(1.0 / pc)
        u0 = f32(2.0) * np.sin(f32(s0c * f32(0.5)), dtype=f32) ** 2
        u1 = f32(2.0) * np.sin(f32(s1c * f32(0.5)), dtype=f32) ** 2
        tre = f32(f32(u1 * s0c) - f32(u0 * s1c)) * qc
        tim = f32(f32(np.sin(s1c, dtype=f32) * s0c)
                  - f32(np.sin(s0c, dtype=f32) * s1c)) * qc
        mask = pc == 0
        tre = np.where(mask, 0.0, tre)
        tim = np.where(mask, 0.0, tim)
        w = cds_w[b].astype(np.float64)
        Fre[b, f] = w @ tre.astype(np.float64)
        Fim[b, f] = w @ tim.astype(np.float64)

    # reference: F[:,0,0] = -sum(CD)/2, then F = -F * RES^2
    dc = CD.sum(axis=1, dtype=np.float64) * (RES**2) / 2.0
    Fre[:, 0] = dc
    Fim[:, 0] = dc

    F = np.stack([Fre, Fim], axis=-1).astype(np.float32)           # (B, NF, 2)
    return F.reshape(B, FX, FY, 1, 2)


if __name__ == "__main__":
    rng = np.random.default_rng(0)
    V = rng.random((B, NV, 2), np.float32)
    idx = np.arange(NV, dtype=np.int32)
    E = np.broadcast_to(
        np.stack([idx, (idx + 1) % NV], -1)[None], (B, NV, 2)
    ).astype(np.int32)
    D = np.ones((B, NV, 1), np.float32)
    out = kernel(V=V, E=E, D=D)
    print(out.shape, out.dtype, np.abs(out).max())

